# revision 63
# baseline (speedup 1.0000x reference)
"""ANI radial symmetry function kernel for 8 TRN2 NeuronCores.

out[b,a,r] = sum_n exp(-etas[r]*(r_ij[b,a,n]-rss[r])**2) * cutoff(r_ij) * mask
  B=16, A=2048, N=96, R=16, cutoff = 0.5*(cos(pi*x/3)+1)*(x<3)

Strategy: each of the 16 radial channels h_r(x) = gauss_r(x)*cutoff(x) is a
smooth 1-D function on x in [0,3].  Substitute z = relu(3-x)*mask (so every
invalid or beyond-cutoff neighbor maps to z=0) and approximate all 16 channels
in a shared basis of NA tanh ridge functions (ScalarE, one ACTIVATE pass each)
plus ND squared-tanh atoms (VectorE products) plus a constant:
  h_r(3-z) ~= C[0,r] + sum_j C[j+1,r] * phi_j(z),        combo(0) ~= 0.
The neighbor reduction and the projection to 16 channels happen in one
PSUM-accumulated TensorE matmul chain with n=96 in the contract dimension;
operands are fp16 (11-bit mantissa, full-rate PE path).  The constant atom
contributes 96*C[0,r] per output, folded into the PSUM->SBUF copy as a
per-partition bias.  C is fit at runtime from the actual etas/rss via
fp16-rounding-aware weighted least squares (numpy).

Sharding: data-parallel over batch, 2 batches per core.  Host pre-transposes
to [B, N, A] so DMA is contiguous with n in partitions.
"""

import os
import sys

import numpy as np

if "/opt/trn_rl_repo" not in sys.path:
    sys.path.insert(0, "/opt/trn_rl_repo")

B, A, N, R = 16, 2048, 96, 16
RC = 3.0
NCORES = 8
BPC = B // NCORES  # batches per core

# tanh basis parameters: phi_j(z) = tanh(a_j*z + b_j) for j < NA (ScalarE),
# phi_{NA+k}(z) = phi_{SQ[k]}(z)^2 for k < ND (VectorE squares).
TANH_A = [1.1022824083472713, 1.115875603032523, 1.2005634868150412,
          -1.2271508013307884]
TANH_B = [-2.4688200491371193, -1.6236457392881667, -3.737631427523937,
          0.7782999978763218]
SQ = [0, 1, 2, 3]
FIT_LAM = 2e-3
NA = len(TANH_A)
ND = len(SQ)
M = NA + ND

AC = 2048  # atom chunk for elementwise tiles
HC = 1024  # half-chunk: PSUM accumulation granularity
MMF = 512  # matmul moving free dim (one PSUM bank of fp32)

_CACHE = {}


def _round_f16(v):
    return np.float16(np.asarray(v, dtype=np.float32)).astype(np.float64)


def _fit_coeffs(etas, rss):
    """fp16-rounding-aware weighted ridge fit of C [M+1, 16] on a z-grid."""
    zg = np.linspace(0.0, RC, 1501)
    xg = RC - zg
    cut = 0.5 * (np.cos(np.pi * xg / RC) + 1.0)
    T = (
        np.exp(-etas[:, None].astype(np.float64) * (xg[None, :] - rss[:, None]) ** 2)
        * cut[None, :]
    )  # [R, Z]
    tan = [_round_f16(np.tanh(a_ * zg + b_)) for a_, b_ in zip(TANH_A, TANH_B)]
    cols = [np.ones_like(zg)] + tan + [_round_f16(tan[k] * tan[k]) for k in SQ]
    Amat = np.stack(cols, axis=1)  # [Z, M+1]
    wgt = np.ones_like(zg)
    wgt[0] = 500.0  # z=0 (masked/out-of-cutoff) must map to ~0
    Aw = Amat * wgt[:, None]
    Areg = np.vstack([Aw, FIT_LAM * np.eye(M + 1)])
    Treg = np.vstack([(T * wgt[None, :]).T, np.zeros((M + 1, T.shape[0]))])
    C, *_ = np.linalg.lstsq(Areg, Treg, rcond=None)  # [M+1, R]
    # compensate for fp16 rounding of C itself (C[0] stays fp32 in the bias)
    Cr = C.copy()
    Cr[1:] = _round_f16(C[1:])
    residw = np.vstack(
        [(T.T - Amat @ Cr) * wgt[:, None], np.zeros((M + 1, T.shape[0]))]
    )
    dC, *_ = np.linalg.lstsq(Areg, residw, rcond=None)
    C2 = Cr + dC
    C2[1:] = _round_f16(C2[1:])
    return C2.astype(np.float32)


def _build_nc():
    import concourse.bass as bass
    import concourse.mybir as mybir
    import concourse.tile as tile
    from concourse import bacc

    f32 = mybir.dt.float32
    f16 = mybir.dt.float16
    AFT = mybir.ActivationFunctionType

    # Skip the Bass-init all-engine barrier (~4us of kernel head): it only
    # guards the const-AP memsets, which this kernel never reads (all
    # activation biases/scales are explicit APs or immediates).
    class _Bacc(bacc.Bacc):
        def all_engine_barrier(self, *a, **kw):
            if not getattr(self, "_skip_init_barrier", True):
                return super().all_engine_barrier(*a, **kw)
            self._skip_init_barrier = False
            return None

    nc = _Bacc("TRN2", target_bir_lowering=False, debug=False,
               enable_asserts=False)
    nc._skip_init_barrier = False
    r_t = nc.dram_tensor("r", [BPC, N, A], f32, kind="ExternalInput")
    cw_t = nc.dram_tensor("cw", [N, M * R], f32, kind="ExternalInput")
    # output: per chunk, 4 column-group atom-quarters (disjoint atoms)
    o_t = nc.dram_tensor(
        "o", [BPC, A // AC, 4, R, MMF], f32, kind="ExternalOutput"
    )

    nch = A // AC

    with tile.TileContext(nc) as tc:
        with (
            tc.tile_pool(name="const", bufs=1) as constp,
            tc.tile_pool(name="io", bufs=4) as iop,
            tc.tile_pool(name="phi", bufs=2) as phip,
            tc.tile_pool(name="psum", bufs=2, space="PSUM") as psump,
            tc.tile_pool(name="outp", bufs=4) as outp,
        ):
            # small consts: keep them off the gpsimd engine (its DRAINs sit
            # in the kernel head) and off the x-load queue head
            cwt_raw = constp.tile([N, M * R], f32)
            nc.gpsimd.dma_start(cwt_raw[:], cw_t[:])
            cwt = constp.tile([N, M * R], f16)
            nc.vector.tensor_copy(cwt[:], cwt_raw[:])
            bvt = constp.tile([N, NA], f32)
            for j in range(NA):
                nc.vector.memset(bvt[:, j:j + 1], float(TANH_B[j]))

            for b in range(BPC):
                for c in range(nch):
                    ci = b * nch + c
                    sl = slice(c * AC, (c + 1) * AC)
                    xt = iop.tile([N, AC], f32, tag="x")
                    # split each load across both HWDGE queues
                    h = AC // 2
                    nc.sync.dma_start(xt[:, 0:h], r_t[b, :, c * AC:c * AC + h])
                    nc.scalar.dma_start(
                        xt[:, h:AC], r_t[b, :, c * AC + h:(c + 1) * AC]
                    )
                    # host pre-fused x' = x + 1e4*(1-mask), so
                    # w = min(x',3) - 3 equals -z for valid neighbors and 0
                    # for invalid/beyond-cutoff ones; tanh atoms use
                    # scale=-a_j so arg = a_j*z + b_j either way.
                    zm = phip.tile([N, AC], f32, tag="zm")
                    nc.vector.tensor_scalar(
                        zm[:], xt[:], 3.0, 3.0,
                        mybir.AluOpType.min, mybir.AluOpType.subtract,
                    )

                    phis = []
                    for j in range(NA):
                        ph = phip.tile([N, AC], f16, tag=f"ph{j}")
                        nc.scalar.activation(
                            ph[:], zm[:], AFT.Tanh,
                            bias=bvt[:, j:j + 1], scale=-float(TANH_A[j]),
                        )
                        phis.append(ph)
                    for k in SQ:
                        ph = phip.tile([N, AC], f16, tag=f"sq{k}")
                        nc.vector.tensor_mul(ph[:], phis[k][:], phis[k][:])
                        phis.append(ph)

                    # 4 column groups of the PE array run concurrently, each
                    # owning one atom-quarter of the chunk and one PSUM bank:
                    # group g accumulates all M basis matmuls for atoms
                    # [g*MMF, (g+1)*MMF) into ps4[32g:32g+16, g*MMF:...].
                    ps4 = psump.tile([128, AC], f32)
                    for j in range(M):
                        for g in range(4):
                            fsl = slice(g * MMF, (g + 1) * MMF)
                            nc.tensor.matmul(
                                ps4[32 * g:32 * g + R, fsl],
                                cwt[:, j * R:(j + 1) * R],
                                phis[j][:, fsl],
                                start=(j == 0), stop=(j == M - 1),
                                tile_position=(0, 32 * g),
                            )
                    ot = outp.tile([128, AC], f32, tag="o")
                    nc.vector.tensor_copy(ot[:], ps4[:])
                    qo = nc.scalar if ci % 2 == 0 else nc.sync
                    for g in range(4):
                        qo.dma_start(
                            o_t[b, c, g],
                            ot[32 * g:32 * g + R, g * MMF:(g + 1) * MMF],
                        )
    nc.compile()
    return nc


def _install_ntff_hook():
    """The slim agent image lacks ``antenv.axon_hooks``; recreate it so
    ``run_bass_kernel_spmd(trace=True)`` can capture NTFF profiles via the
    axon PJRT plugin's nrt-profile C ABI (same mechanism as trn_boot)."""
    import types

    try:
        import antenv.axon_hooks  # noqa: F401
        return
    except ImportError:
        pass
    try:
        import antenv
        from trn_agent_boot.trn_boot import _ntff_profile_via_ctypes
    except ImportError:
        return
    holder = {}
    mod = types.ModuleType("antenv.axon_hooks")
    mod.set_axon_ntff_profile_hook = lambda h: holder.__setitem__("h", h)
    mod.get_axon_ntff_profile_hook = lambda: holder.get("h")
    sys.modules["antenv.axon_hooks"] = mod
    antenv.axon_hooks = mod
    hook = _ntff_profile_via_ctypes("/opt/axon/libaxon_pjrt.so")
    if hook is not None:
        mod.set_axon_ntff_profile_hook(hook)
    # artifact upload needs S3 creds the container doesn't have
    from concourse import bass_utils as _bu

    _bu.upload_artifacts = lambda tmpdir: tmpdir


def kernel(r_ij, mask, etas, rss):
    from concourse.bass_utils import run_bass_kernel_spmd

    if os.environ.get("BASS_TRACE"):
        _install_ntff_hook()

    r_ij = np.asarray(r_ij, dtype=np.float32)
    mask = np.asarray(mask, dtype=np.float32)
    etas = np.asarray(etas, dtype=np.float32)
    rss = np.asarray(rss, dtype=np.float32)

    C = _fit_coeffs(etas, rss)  # [M+1, R]; row 0 = constant atom
    cw = np.ascontiguousarray(
        np.broadcast_to(C[1:].reshape(1, M * R), (N, M * R))
    ).astype(np.float32)

    # host-side: fuse the validity mask into x (invalid -> x'=1e4 maps to
    # z=0 on device) and transpose to [B, N, A] so n lands in the
    # partition dim
    xf = r_ij + np.float32(1e4) * (np.float32(1.0) - mask)
    xT = np.ascontiguousarray(xf.transpose(0, 2, 1))

    if "nc" not in _CACHE:
        _CACHE["nc"] = _build_nc()
    nc = _CACHE["nc"]

    in_maps = [
        {
            "r": np.ascontiguousarray(xT[i * BPC:(i + 1) * BPC]),
            "cw": cw,
        }
        for i in range(NCORES)
    ]
    res = run_bass_kernel_spmd(
        nc, in_maps, core_ids=list(range(NCORES)),
        trace=bool(os.environ.get("BASS_TRACE")),
    )
    global LAST_RESULT
    LAST_RESULT = res

    out = np.concatenate([res.results[i]["o"] for i in range(NCORES)], axis=0)
    # [B, nch, 4, R, MMF]: atom index a = c*AC + g*MMF + f; add the
    # constant atom and rearrange to [B, A, R]
    out = out + (N * C[0])[None, None, None, :, None]
    out = out.transpose(0, 1, 2, 4, 3).reshape(B, A, R)
    return np.ascontiguousarray(out).astype(np.float32)


LAST_RESULT = None


# revision 67
# speedup vs baseline: 1.1057x; 1.1057x over previous
"""ANI radial symmetry function kernel for 8 TRN2 NeuronCores.

out[b,a,r] = sum_n exp(-etas[r]*(r_ij[b,a,n]-rss[r])**2) * cutoff(r_ij) * mask
  B=16, A=2048, N=96, R=16, cutoff = 0.5*(cos(pi*x/3)+1)*(x<3)

Strategy: each of the 16 radial channels h_r(x) = gauss_r(x)*cutoff(x) is a
smooth 1-D function on x in [0,3].  Substitute z = relu(3-x)*mask (so every
invalid or beyond-cutoff neighbor maps to z=0) and approximate all 16 channels
in a shared basis of NA tanh ridge functions (ScalarE, one ACTIVATE pass each)
plus ND squared-tanh atoms (VectorE products) plus a constant:
  h_r(3-z) ~= C[0,r] + sum_j C[j+1,r] * phi_j(z),        combo(0) ~= 0.
The neighbor reduction and the projection to 16 channels happen in one
PSUM-accumulated TensorE matmul chain with n=96 in the contract dimension;
operands are fp16 (11-bit mantissa, full-rate PE path).  The constant atom
contributes 96*C[0,r] per output, folded into the PSUM->SBUF copy as a
per-partition bias.  C is fit at runtime from the actual etas/rss via
fp16-rounding-aware weighted least squares (numpy).

Sharding: data-parallel over batch, 2 batches per core.  Host pre-transposes
to [B, N, A] so DMA is contiguous with n in partitions.
"""

import os
import sys

import numpy as np

if "/opt/trn_rl_repo" not in sys.path:
    sys.path.insert(0, "/opt/trn_rl_repo")

B, A, N, R = 16, 2048, 96, 16
RC = 3.0
NCORES = 8
BPC = B // NCORES  # batches per core

# tanh basis parameters: phi_j(z) = tanh(a_j*z + b_j) for j < NA (ScalarE),
# phi_{NA+k}(z) = phi_{SQ[k]}(z)^2 for k < ND (VectorE squares).
TANH_A = [1.1022824083472713, 1.115875603032523, 1.2005634868150412,
          -1.2271508013307884]
TANH_B = [-2.4688200491371193, -1.6236457392881667, -3.737631427523937,
          0.7782999978763218]
SQ = [0, 1, 2, 3]
FIT_LAM = 2e-3
NA = len(TANH_A)
ND = len(SQ)
M = NA + ND

AC = 2048  # atom chunk for elementwise tiles
HC = 1024  # half-chunk: PSUM accumulation granularity
MMF = 512  # matmul moving free dim (one PSUM bank of fp32)

_CACHE = {}


def _round_f16(v):
    return np.float16(np.asarray(v, dtype=np.float32)).astype(np.float64)


def _fit_coeffs(etas, rss):
    """fp16-rounding-aware weighted ridge fit of C [M+1, 16] on a z-grid."""
    zg = np.linspace(0.0, RC, 1501)
    xg = RC - zg
    cut = 0.5 * (np.cos(np.pi * xg / RC) + 1.0)
    T = (
        np.exp(-etas[:, None].astype(np.float64) * (xg[None, :] - rss[:, None]) ** 2)
        * cut[None, :]
    )  # [R, Z]
    tan = [_round_f16(np.tanh(a_ * zg + b_)) for a_, b_ in zip(TANH_A, TANH_B)]
    cols = [np.ones_like(zg)] + tan + [_round_f16(tan[k] * tan[k]) for k in SQ]
    Amat = np.stack(cols, axis=1)  # [Z, M+1]
    wgt = np.ones_like(zg)
    wgt[0] = 500.0  # z=0 (masked/out-of-cutoff) must map to ~0
    Aw = Amat * wgt[:, None]
    Areg = np.vstack([Aw, FIT_LAM * np.eye(M + 1)])
    Treg = np.vstack([(T * wgt[None, :]).T, np.zeros((M + 1, T.shape[0]))])
    C, *_ = np.linalg.lstsq(Areg, Treg, rcond=None)  # [M+1, R]
    # compensate for fp16 rounding of C itself (C[0] stays fp32 in the bias)
    Cr = C.copy()
    Cr[1:] = _round_f16(C[1:])
    residw = np.vstack(
        [(T.T - Amat @ Cr) * wgt[:, None], np.zeros((M + 1, T.shape[0]))]
    )
    dC, *_ = np.linalg.lstsq(Areg, residw, rcond=None)
    C2 = Cr + dC
    C2[1:] = _round_f16(C2[1:])
    return C2.astype(np.float32)


def _build_nc():
    import concourse.bass as bass
    import concourse.mybir as mybir
    import concourse.tile as tile
    from concourse import bacc

    f32 = mybir.dt.float32
    f16 = mybir.dt.float16
    AFT = mybir.ActivationFunctionType

    # Skip the Bass-init all-engine barrier (~4us of kernel head): it only
    # guards the const-AP memsets, which this kernel never reads (all
    # activation biases/scales are explicit APs or immediates).
    class _Bacc(bacc.Bacc):
        def all_engine_barrier(self, *a, **kw):
            if not getattr(self, "_skip_init_barrier", True):
                return super().all_engine_barrier(*a, **kw)
            self._skip_init_barrier = False
            return None

    nc = _Bacc("TRN2", target_bir_lowering=False, debug=False,
               enable_asserts=False)
    nc._skip_init_barrier = False
    r_t = nc.dram_tensor("r", [BPC, N, A], f32, kind="ExternalInput")
    cw_t = nc.dram_tensor("cw", [N, M * R], f32, kind="ExternalInput")
    o_t = nc.dram_tensor("o", [BPC, R, A], f32, kind="ExternalOutput")

    # smaller first chunk so the pipeline starts sooner
    chunk_list = [(0, 0, 1024), (0, 1024, 1024), (1, 0, 2048)]
    chunk_list = [cl for cl in chunk_list if cl[0] < BPC]

    with tile.TileContext(nc) as tc:
        with (
            tc.tile_pool(name="const", bufs=1) as constp,
            tc.tile_pool(name="io", bufs=4) as iop,
            tc.tile_pool(name="phi", bufs=2) as phip,
            tc.tile_pool(name="psum", bufs=2, space="PSUM") as psump,
            tc.tile_pool(name="outp", bufs=4) as outp,
        ):
            # small consts: keep them off the gpsimd engine (its DRAINs sit
            # in the kernel head) and off the x-load queue head
            cwt_raw = constp.tile([N, M * R], f32)
            nc.gpsimd.dma_start(cwt_raw[:], cw_t[:])
            cwt = constp.tile([N, M * R], f16)
            nc.vector.tensor_copy(cwt[:], cwt_raw[:])
            bvt = constp.tile([N, NA], f32)
            for j in range(NA):
                nc.vector.memset(bvt[:, j:j + 1], float(TANH_B[j]))

            for ci, (b, off, sz) in enumerate(chunk_list):
                    q = sz // 4
                    xt = iop.tile([N, sz], f32, tag="x")
                    # split each load across both HWDGE queues
                    h = sz // 2
                    nc.sync.dma_start(xt[:, 0:h], r_t[b, :, off:off + h])
                    nc.scalar.dma_start(
                        xt[:, h:sz], r_t[b, :, off + h:off + sz]
                    )
                    # host pre-fused x' = x + 1e4*(1-mask), so
                    # w = min(x',3) - 3 equals -z for valid neighbors and 0
                    # for invalid/beyond-cutoff ones; tanh atoms use
                    # scale=-a_j so arg = a_j*z + b_j either way.
                    zm = phip.tile([N, sz], f32, tag="zm")
                    nc.vector.tensor_scalar(
                        zm[:], xt[:], 3.0, 3.0,
                        mybir.AluOpType.min, mybir.AluOpType.subtract,
                    )

                    phis = []
                    for j in range(NA):
                        ph = phip.tile([N, sz], f16, tag=f"ph{j}")
                        nc.scalar.activation(
                            ph[:], zm[:], AFT.Tanh,
                            bias=bvt[:, j:j + 1], scale=-float(TANH_A[j]),
                        )
                        phis.append(ph)
                    for k in SQ:
                        ph = phip.tile([N, sz], f16, tag=f"sq{k}")
                        nc.vector.tensor_mul(ph[:], phis[k][:], phis[k][:])
                        phis.append(ph)

                    # 4 column groups of the PE array run concurrently, each
                    # owning one atom-quarter of the chunk and one PSUM bank:
                    # group g accumulates all M basis matmuls for atoms
                    # [g*q, (g+1)*q) into ps4[32g:32g+16, g*q:...].
                    ps4 = psump.tile([128, sz], f32)
                    for j in range(M):
                        for g in range(4):
                            fsl = slice(g * q, (g + 1) * q)
                            nc.tensor.matmul(
                                ps4[32 * g:32 * g + R, fsl],
                                cwt[:, j * R:(j + 1) * R],
                                phis[j][:, fsl],
                                start=(j == 0), stop=(j == M - 1),
                                tile_position=(0, 32 * g),
                            )
                    ot = outp.tile([128, sz // 4], f32, tag="o")
                    for g in range(4):
                        src = ps4[32 * g:32 * g + R, g * q:(g + 1) * q]
                        dst = ot[32 * g:32 * g + R, :]
                        if g % 2 == 0:
                            nc.vector.tensor_copy(dst, src)
                        else:
                            nc.scalar.copy(dst, src)
                    qo = nc.scalar if ci % 2 == 0 else nc.sync
                    og = ot[:].rearrange("(g k) f -> g k f", g=4)[:, 0:R, :]
                    od = o_t[b, :, off:off + sz].rearrange(
                        "r (g f) -> g r f", g=4
                    )
                    qo.dma_start(od, og)
    nc.compile()
    return nc


def _install_ntff_hook():
    """The slim agent image lacks ``antenv.axon_hooks``; recreate it so
    ``run_bass_kernel_spmd(trace=True)`` can capture NTFF profiles via the
    axon PJRT plugin's nrt-profile C ABI (same mechanism as trn_boot)."""
    import types

    try:
        import antenv.axon_hooks  # noqa: F401
        return
    except ImportError:
        pass
    try:
        import antenv
        from trn_agent_boot.trn_boot import _ntff_profile_via_ctypes
    except ImportError:
        return
    holder = {}
    mod = types.ModuleType("antenv.axon_hooks")
    mod.set_axon_ntff_profile_hook = lambda h: holder.__setitem__("h", h)
    mod.get_axon_ntff_profile_hook = lambda: holder.get("h")
    sys.modules["antenv.axon_hooks"] = mod
    antenv.axon_hooks = mod
    hook = _ntff_profile_via_ctypes("/opt/axon/libaxon_pjrt.so")
    if hook is not None:
        mod.set_axon_ntff_profile_hook(hook)
    # artifact upload needs S3 creds the container doesn't have
    from concourse import bass_utils as _bu

    _bu.upload_artifacts = lambda tmpdir: tmpdir


def kernel(r_ij, mask, etas, rss):
    from concourse.bass_utils import run_bass_kernel_spmd

    if os.environ.get("BASS_TRACE"):
        _install_ntff_hook()

    r_ij = np.asarray(r_ij, dtype=np.float32)
    mask = np.asarray(mask, dtype=np.float32)
    etas = np.asarray(etas, dtype=np.float32)
    rss = np.asarray(rss, dtype=np.float32)

    C = _fit_coeffs(etas, rss)  # [M+1, R]; row 0 = constant atom
    cw = np.ascontiguousarray(
        np.broadcast_to(C[1:].reshape(1, M * R), (N, M * R))
    ).astype(np.float32)

    # host-side: fuse the validity mask into x (invalid -> x'=1e4 maps to
    # z=0 on device) and transpose to [B, N, A] so n lands in the
    # partition dim
    xf = r_ij + np.float32(1e4) * (np.float32(1.0) - mask)
    xT = np.ascontiguousarray(xf.transpose(0, 2, 1))

    if "nc" not in _CACHE:
        _CACHE["nc"] = _build_nc()
    nc = _CACHE["nc"]

    in_maps = [
        {
            "r": np.ascontiguousarray(xT[i * BPC:(i + 1) * BPC]),
            "cw": cw,
        }
        for i in range(NCORES)
    ]
    res = run_bass_kernel_spmd(
        nc, in_maps, core_ids=list(range(NCORES)),
        trace=bool(os.environ.get("BASS_TRACE")),
    )
    global LAST_RESULT
    LAST_RESULT = res

    out = np.concatenate([res.results[i]["o"] for i in range(NCORES)], axis=0)
    # [B, R, A]: add the constant atom and transpose to [B, A, R]
    out = out + (N * C[0])[None, :, None]
    return np.ascontiguousarray(out.transpose(0, 2, 1)).astype(np.float32)


LAST_RESULT = None
